# revision 1
# baseline (speedup 1.0000x reference)
"""GraphTransformerBlock (PyG TransformerConv + FFN block) on 8 trn2 cores.

Sharding: nodes partitioned into 8 contiguous ranges of 6250 (dst owner).
Edges sorted by dst on host, assigned to the owner core of their dst,
grouped into 49 windows of 128 dst nodes per core, padded to a uniform
number of 128-edge tiles per window (TW, data-dependent, computed on host).

Per edge tile (128 edges) on device:
  - indirect-gather x[src] rows [128,128], PE-transpose
  - one PSUM accum group: kv = xT.T @ [Wk.T|Wv.T]  +  [ea|1].T @ [[We.T|We.T];[bk|bv]]
    -> kv_ps = [k_j || m] = [k[src]+e+bk || v[src]+e+bv]
  - indirect-gather q[dst] rows [128,256] from per-core q table
  - alpha_h = sum(q * k_j) per head (fused tensor_tensor_reduce), ex = exp(alpha/sqrt(C))
  - rhs = [m*ex_h || ex]  [128,258]
  - one-hot(dst within window) @ rhs accumulated into window PSUM [128,258]
Per window epilogue: normalize by denom (softmax normalization commutes with
the segment sum), mean heads, +skip matmul, LN1, FFN (leaky relu via max),
LN2, write out.
"""

import math

import numpy as np

import concourse.bass as bass
import concourse.bacc as bacc
import concourse.mybir as mybir
from concourse.tile import TileContext
from concourse.bass_utils import run_bass_kernel_spmd

N, E, HID, EDIM, H, C = 50000, 400000, 128, 32, 2, 128
M = 8            # cores
NPC = N // M     # 6250 nodes per core
P = 128
NW = math.ceil(NPC / P)      # 49 windows per core
NPAD = NW * P                # 6272
NFULL = math.ceil(N / P) * P # 50048
F32 = mybir.dt.float32
I32 = mybir.dt.int32
INV_SQRT_C = 1.0 / math.sqrt(C)

_cache: dict[int, "bacc.Bacc"] = {}


def _build(TW: int) -> "bacc.Bacc":
    if TW in _cache:
        return _cache[TW]
    NT = NW * TW            # edge tiles per core
    EPC = NT * P            # padded edges per core

    nc = bacc.Bacc("TRN2", target_bir_lowering=False)
    x_full = nc.declare_dram_parameter("x_full", [NFULL, HID], F32, isOutput=False)
    x_own = nc.declare_dram_parameter("x_own", [NPAD, HID], F32, isOutput=False)
    eaT = nc.declare_dram_parameter("eaT", [EDIM + 1, EPC], F32, isOutput=False)
    srcQ = nc.declare_dram_parameter("srcQ", [P, NT], I32, isOutput=False)
    dstQ = nc.declare_dram_parameter("dstQ", [P, NT], I32, isOutput=False)
    dstW = nc.declare_dram_parameter("dstW", [P, NT], F32, isOutput=False)
    WkvT = nc.declare_dram_parameter("WkvT", [HID, 2 * H * C], F32, isOutput=False)
    WeT2b = nc.declare_dram_parameter("WeT2b", [EDIM + 1, 2 * H * C], F32, isOutput=False)
    WqT = nc.declare_dram_parameter("WqT", [HID, H * C], F32, isOutput=False)
    bq_row = nc.declare_dram_parameter("bq_row", [1, H * C], F32, isOutput=False)
    WskipT = nc.declare_dram_parameter("WskipT", [HID, HID], F32, isOutput=False)
    bskip_b = nc.declare_dram_parameter("bskip_b", [P, HID], F32, isOutput=False)
    W1T = nc.declare_dram_parameter("W1T", [HID, 2 * HID], F32, isOutput=False)
    b1_b = nc.declare_dram_parameter("b1_b", [P, 2 * HID], F32, isOutput=False)
    W2Ta = nc.declare_dram_parameter("W2Ta", [HID, HID], F32, isOutput=False)
    W2Tb = nc.declare_dram_parameter("W2Tb", [HID, HID], F32, isOutput=False)
    b2_b = nc.declare_dram_parameter("b2_b", [P, HID], F32, isOutput=False)
    ln1g = nc.declare_dram_parameter("ln1g", [P, HID], F32, isOutput=False)
    ln1b = nc.declare_dram_parameter("ln1b", [P, HID], F32, isOutput=False)
    ln2g = nc.declare_dram_parameter("ln2g", [P, HID], F32, isOutput=False)
    ln2b = nc.declare_dram_parameter("ln2b", [P, HID], F32, isOutput=False)
    ident = nc.declare_dram_parameter("ident", [P, P], F32, isOutput=False)
    iota = nc.declare_dram_parameter("iota", [P, P], F32, isOutput=False)
    ones_col = nc.declare_dram_parameter("ones_col", [1, P], F32, isOutput=False)
    out = nc.declare_dram_parameter("out", [NPAD, HID], F32, isOutput=True)

    q_table = nc.dram_tensor("q_table", [NPAD, H * C], F32)

    add = mybir.AluOpType.add
    mult = mybir.AluOpType.mult
    is_equal = mybir.AluOpType.is_equal
    op_max = mybir.AluOpType.max
    subtract = mybir.AluOpType.subtract
    AX = mybir.AxisListType.X
    Exp = mybir.ActivationFunctionType.Exp
    Sqrt = mybir.ActivationFunctionType.Sqrt

    with TileContext(nc) as tc:
        with (
            tc.tile_pool(name="const", bufs=1) as cp,
            tc.tile_pool(name="ea", bufs=2) as eap,
            tc.tile_pool(name="edge", bufs=3) as ep,
            tc.tile_pool(name="node", bufs=2) as np_,
            tc.tile_pool(name="ps_kv", bufs=2, space="PSUM") as ps_kv,
            tc.tile_pool(name="ps_agg", bufs=2, space="PSUM") as ps_agg,
            tc.tile_pool(name="ps_t", bufs=2, space="PSUM") as ps_t,
            tc.tile_pool(name="ps_misc", bufs=2, space="PSUM") as ps_misc,
        ):
            WkvT_s = cp.tile_from(WkvT[:])
            WeT2b_s = cp.tile_from(WeT2b[:])
            WqT_s = cp.tile_from(WqT[:])
            bq_s = cp.tile_from(bq_row[:])
            WskipT_s = cp.tile_from(WskipT[:])
            bskip_s = cp.tile_from(bskip_b[:])
            W1T_s = cp.tile_from(W1T[:])
            b1_s = cp.tile_from(b1_b[:])
            W2Ta_s = cp.tile_from(W2Ta[:])
            W2Tb_s = cp.tile_from(W2Tb[:])
            b2_s = cp.tile_from(b2_b[:])
            ln1g_s = cp.tile_from(ln1g[:])
            ln1b_s = cp.tile_from(ln1b[:])
            ln2g_s = cp.tile_from(ln2g[:])
            ln2b_s = cp.tile_from(ln2b[:])
            ident_s = cp.tile_from(ident[:])
            iota_s = cp.tile_from(iota[:])
            ones_s = cp.tile_from(ones_col[:])
            srcQ_s = cp.tile_from(srcQ[:])
            dstQ_s = cp.tile_from(dstQ[:])
            dstW_s = cp.tile_from(dstW[:])
            eps_s = cp.tile([P, 1], F32)
            nc.vector.memset(eps_s[:], 1e-5)

            # ---- Phase A: q table (own node range) ----
            for w in range(NW):
                xw = np_.tile([P, HID], F32, tag="xw_a")
                nc.sync.dma_start(out=xw[:], in_=x_own[w * P:(w + 1) * P, :])
                xwT_ps = ps_t.tile([P, P], F32, tag="pst")
                nc.tensor.transpose(out=xwT_ps[:], in_=xw[:], identity=ident_s[:])
                xwT = np_.tile([P, P], F32, tag="xwT_a")
                nc.vector.tensor_copy(out=xwT[:], in_=xwT_ps[:])
                q_ps = ps_misc.tile([P, H * C], F32, tag="psm")
                nc.tensor.matmul(out=q_ps[:], lhsT=xwT[:], rhs=WqT_s[:],
                                 start=True, stop=False)
                nc.tensor.matmul(out=q_ps[:], lhsT=ones_s[:], rhs=bq_s[:],
                                 start=False, stop=True)
                qsb = np_.tile([P, H * C], F32, tag="qsb_a")
                nc.vector.tensor_copy(out=qsb[:], in_=q_ps[:])
                nc.sync.dma_start(out=q_table[w * P:(w + 1) * P, :], in_=qsb[:])

            # ---- Phase B: edge loop + per-window epilogue ----
            for w in range(NW):
                ea_ch = eap.tile([EDIM + 1, TW * P], F32, tag="ea_ch")
                nc.sync.dma_start(out=ea_ch[:], in_=eaT[:, w * TW * P:(w + 1) * TW * P])
                agg_ps = ps_agg.tile([P, H * C + H], F32)
                for t in range(TW):
                    g = w * TW + t
                    xs = ep.tile([P, HID], F32, tag="xs")
                    nc.gpsimd.indirect_dma_start(
                        out=xs[:], out_offset=None, in_=x_full[:],
                        in_offset=bass.IndirectOffsetOnAxis(ap=srcQ_s[:, g:g + 1], axis=0))
                    qr = ep.tile([P, H * C], F32, tag="qr")
                    nc.gpsimd.indirect_dma_start(
                        out=qr[:], out_offset=None, in_=q_table[:],
                        in_offset=bass.IndirectOffsetOnAxis(ap=dstQ_s[:, g:g + 1], axis=0))
                    xsT_ps = ps_t.tile([P, P], F32, tag="pst")
                    nc.tensor.transpose(out=xsT_ps[:], in_=xs[:], identity=ident_s[:])
                    xsT = ep.tile([P, P], F32, tag="xsT")
                    nc.vector.tensor_copy(out=xsT[:], in_=xsT_ps[:])
                    kv_ps = ps_kv.tile([P, 2 * H * C], F32)
                    nc.tensor.matmul(out=kv_ps[:], lhsT=xsT[:], rhs=WkvT_s[:],
                                     start=True, stop=False)
                    nc.tensor.matmul(out=kv_ps[:], lhsT=ea_ch[:, t * P:(t + 1) * P],
                                     rhs=WeT2b_s[:], start=False, stop=True)
                    # alpha per head: elementwise mul then free-axis reduce
                    # (tensor_tensor_reduce wedges the device on this runtime)
                    scr = ep.tile([P, H * C], F32, tag="scr")
                    alpha = ep.tile([P, H], F32, tag="alpha")
                    nc.vector.tensor_tensor(out=scr[:], in0=qr[:],
                                            in1=kv_ps[:, 0:H * C], op=mult)
                    for h in range(H):
                        nc.vector.tensor_reduce(
                            out=alpha[:, h:h + 1], in_=scr[:, h * C:(h + 1) * C],
                            axis=AX, op=add)
                    ex = ep.tile([P, H], F32, tag="ex")
                    nc.scalar.activation(out=ex[:], in_=alpha[:], func=Exp,
                                         scale=INV_SQRT_C)
                    rhs = ep.tile([P, H * C + H], F32, tag="rhs")
                    for h in range(H):
                        nc.vector.tensor_scalar_mul(
                            rhs[:, h * C:(h + 1) * C],
                            kv_ps[:, (H + h) * C:(H + h + 1) * C],
                            ex[:, h:h + 1])
                    nc.vector.tensor_copy(out=rhs[:, H * C:H * C + H], in_=ex[:])
                    oh = ep.tile([P, P], F32, tag="oh")
                    nc.vector.tensor_scalar(
                        out=oh[:], in0=iota_s[:], scalar1=dstW_s[:, g:g + 1],
                        scalar2=None, op0=is_equal)
                    nc.tensor.matmul(out=agg_ps[:], lhsT=oh[:], rhs=rhs[:],
                                     start=(t == 0), stop=(t == TW - 1))

                # ---- epilogue for window w ----
                xw = np_.tile([P, HID], F32, tag="xw")
                nc.sync.dma_start(out=xw[:], in_=x_own[w * P:(w + 1) * P, :])
                xwT_ps = ps_t.tile([P, P], F32, tag="pst")
                nc.tensor.transpose(out=xwT_ps[:], in_=xw[:], identity=ident_s[:])
                xwT = np_.tile([P, P], F32, tag="xwT")
                nc.vector.tensor_copy(out=xwT[:], in_=xwT_ps[:])
                sk_ps = ps_misc.tile([P, HID], F32, tag="psm")
                nc.tensor.matmul(out=sk_ps[:], lhsT=xwT[:], rhs=WskipT_s[:],
                                 start=True, stop=True)
                den = np_.tile([P, H], F32, tag="den")
                nc.vector.tensor_scalar_add(den[:], agg_ps[:, H * C:H * C + H], 1e-16)
                rcp = np_.tile([P, H], F32, tag="rcp")
                nc.vector.reciprocal(out=rcp[:], in_=den[:])
                a0 = np_.tile([P, HID], F32, tag="a0")
                a1 = np_.tile([P, HID], F32, tag="a1")
                nc.vector.tensor_scalar_mul(a0[:], agg_ps[:, 0:C], rcp[:, 0:1])
                nc.vector.tensor_scalar_mul(a1[:], agg_ps[:, C:2 * C], rcp[:, 1:2])
                pre = np_.tile([P, HID], F32, tag="pre")
                nc.vector.tensor_add(out=pre[:], in0=a0[:], in1=a1[:])
                nc.vector.tensor_scalar_mul(pre[:], pre[:], 0.5)
                nc.vector.tensor_add(out=pre[:], in0=pre[:], in1=sk_ps[:])
                nc.vector.tensor_add(out=pre[:], in0=pre[:], in1=bskip_s[:])
                nc.vector.tensor_add(out=pre[:], in0=pre[:], in1=xw[:])

                hh = np_.tile([P, HID], F32, tag="hh")
                _layernorm(nc, np_, pre, hh, ln1g_s, ln1b_s, eps_s, Sqrt, AX, add,
                           subtract, mult, w, "ln1")

                # FFN
                hhT_ps = ps_t.tile([P, P], F32, tag="pst")
                nc.tensor.transpose(out=hhT_ps[:], in_=hh[:], identity=ident_s[:])
                hhT = np_.tile([P, P], F32, tag="hhT")
                nc.vector.tensor_copy(out=hhT[:], in_=hhT_ps[:])
                f1_ps = ps_misc.tile([P, 2 * HID], F32, tag="psm")
                nc.tensor.matmul(out=f1_ps[:], lhsT=hhT[:], rhs=W1T_s[:],
                                 start=True, stop=True)
                f1 = np_.tile([P, 2 * HID], F32, tag="f1")
                nc.vector.tensor_add(out=f1[:], in0=f1_ps[:], in1=b1_s[:])
                # leaky_relu(z) = max(z, 0.01*z)  (DVE only, no ACT table switch)
                f1s = np_.tile([P, 2 * HID], F32, tag="f1s")
                nc.vector.tensor_scalar_mul(f1s[:], f1[:], 0.01)
                nc.vector.tensor_tensor(out=f1[:], in0=f1[:], in1=f1s[:], op=op_max)
                f1T = np_.tile([P, 2 * HID], F32, tag="f1T")
                for hhalf in range(2):
                    fT_ps = ps_t.tile([P, P], F32, tag="pst")
                    nc.tensor.transpose(out=fT_ps[:], in_=f1[:, hhalf * P:(hhalf + 1) * P],
                                        identity=ident_s[:])
                    nc.vector.tensor_copy(out=f1T[:, hhalf * P:(hhalf + 1) * P],
                                          in_=fT_ps[:])
                f2_ps = ps_misc.tile([P, HID], F32, tag="psm")
                nc.tensor.matmul(out=f2_ps[:], lhsT=f1T[:, 0:P], rhs=W2Ta_s[:],
                                 start=True, stop=False)
                nc.tensor.matmul(out=f2_ps[:], lhsT=f1T[:, P:2 * P], rhs=W2Tb_s[:],
                                 start=False, stop=True)
                pre2 = np_.tile([P, HID], F32, tag="pre2")
                nc.vector.tensor_add(out=pre2[:], in0=f2_ps[:], in1=b2_s[:])
                nc.vector.tensor_add(out=pre2[:], in0=pre2[:], in1=hh[:])
                ow = np_.tile([P, HID], F32, tag="ow")
                _layernorm(nc, np_, pre2, ow, ln2g_s, ln2b_s, eps_s, Sqrt, AX, add,
                           subtract, mult, w, "ln2")
                nc.sync.dma_start(out=out[w * P:(w + 1) * P, :], in_=ow[:])

    nc.compile()
    _cache[TW] = nc
    return nc


def _layernorm(nc, pool, x_in, x_out, g_s, b_s, eps_s, Sqrt, AX, add, subtract,
               mult, w, tag):
    P_, D = x_in.shape[0], x_in.shape[1]
    mu = pool.tile([P_, 1], mybir.dt.float32, tag=tag + "_mu")
    nc.vector.tensor_reduce(out=mu[:], in_=x_in[:], axis=AX, op=add)
    nc.vector.tensor_scalar_mul(mu[:], mu[:], 1.0 / D)
    cen = pool.tile([P_, D], mybir.dt.float32, tag=tag + "_cen")
    nc.vector.tensor_scalar(out=cen[:], in0=x_in[:], scalar1=mu[:, 0:1],
                            scalar2=None, op0=subtract)
    sq = pool.tile([P_, D], mybir.dt.float32, tag=tag + "_sq")
    nc.vector.tensor_tensor(out=sq[:], in0=cen[:], in1=cen[:], op=mult)
    vs = pool.tile([P_, 1], mybir.dt.float32, tag=tag + "_vs")
    nc.vector.tensor_reduce(out=vs[:], in_=sq[:], axis=AX, op=add)
    sd = pool.tile([P_, 1], mybir.dt.float32, tag=tag + "_sd")
    nc.scalar.activation(out=sd[:], in_=vs[:], func=Sqrt, scale=1.0 / D,
                         bias=eps_s[:, 0:1])
    rstd = pool.tile([P_, 1], mybir.dt.float32, tag=tag + "_rstd")
    nc.vector.reciprocal(out=rstd[:], in_=sd[:])
    nc.vector.tensor_scalar(out=x_out[:], in0=cen[:], scalar1=rstd[:, 0:1],
                            scalar2=None, op0=mult)
    nc.vector.tensor_tensor(out=x_out[:], in0=x_out[:], in1=g_s[:], op=mult)
    nc.vector.tensor_tensor(out=x_out[:], in0=x_out[:], in1=b_s[:], op=add)


def _prep(inputs):
    x = np.asarray(inputs["x"], np.float32)
    ei = np.asarray(inputs["edge_index"])
    ea = np.asarray(inputs["edge_attr"], np.float32)
    src = ei[0].astype(np.int64)
    dst = ei[1].astype(np.int64)
    order = np.argsort(dst, kind="stable")
    src_s = src[order].astype(np.int32)
    dst_s = dst[order]
    ea_s = ea[order]

    core = (dst_s // NPC).astype(np.int64)
    local = (dst_s - core * NPC).astype(np.int32)
    winl = local // P
    dwin = (local % P).astype(np.float32)
    gid = core * NW + winl                      # non-decreasing
    starts = np.searchsorted(gid, np.arange(M * NW), side="left")
    rank = np.arange(E) - starts[gid]
    counts = np.bincount(gid, minlength=M * NW)
    TW = max(1, int(math.ceil(counts.max() / P)))
    EPC = NW * TW * P
    slot = winl.astype(np.int64) * (TW * P) + rank

    srcA = np.zeros((M, EPC), np.int32)
    dstQA = np.zeros((M, EPC), np.int32)
    dstWA = np.full((M, EPC), -1.0, np.float32)
    eaA = np.zeros((M, EPC, EDIM), np.float32)
    srcA[core, slot] = src_s
    dstQA[core, slot] = local
    dstWA[core, slot] = dwin
    eaA[core, slot] = ea_s

    x_full = np.zeros((NFULL, HID), np.float32)
    x_full[:N] = x
    ident = np.eye(P, dtype=np.float32)
    iota = np.broadcast_to(np.arange(P, dtype=np.float32), (P, P)).copy()

    def bb(v, width):  # broadcast bias to [P, width]
        return np.broadcast_to(np.asarray(v, np.float32), (P, width)).copy()

    Wk, Wv, Wq, We = (np.asarray(inputs[k], np.float32) for k in ("Wk", "Wv", "Wq", "We"))
    WkvT = np.concatenate([Wk.T, Wv.T], axis=1).copy()
    WeT2 = np.concatenate([We.T, We.T], axis=1)
    bkv = np.concatenate([np.asarray(inputs["bk"], np.float32),
                          np.asarray(inputs["bv"], np.float32)])[None, :]
    WeT2b = np.concatenate([WeT2, bkv], axis=0).copy()
    W2T = np.asarray(inputs["W2"], np.float32).T.copy()

    common = dict(
        x_full=x_full,
        WkvT=WkvT, WeT2b=WeT2b,
        WqT=Wq.T.copy(), bq_row=np.asarray(inputs["bq"], np.float32)[None, :].copy(),
        WskipT=np.asarray(inputs["Wskip"], np.float32).T.copy(),
        bskip_b=bb(inputs["bskip"], HID),
        W1T=np.asarray(inputs["W1"], np.float32).T.copy(),
        b1_b=bb(inputs["b1"], 2 * HID),
        W2Ta=W2T[:HID].copy(), W2Tb=W2T[HID:].copy(),
        b2_b=bb(inputs["b2"], HID),
        ln1g=bb(inputs["ln1_g"], HID), ln1b=bb(inputs["ln1_b"], HID),
        ln2g=bb(inputs["ln2_g"], HID), ln2b=bb(inputs["ln2_b"], HID),
        ident=ident, iota=iota, ones_col=np.ones((1, P), np.float32),
    )

    in_maps = []
    for c in range(M):
        x_own = np.zeros((NPAD, HID), np.float32)
        x_own[:NPC] = x[c * NPC:(c + 1) * NPC]
        eaT_c = np.concatenate(
            [eaA[c].T, np.ones((1, EPC), np.float32)], axis=0).copy()
        NT = NW * TW
        in_maps.append(dict(
            common,
            x_own=x_own,
            eaT=eaT_c,
            srcQ=srcA[c].reshape(NT, P).T.copy(),
            dstQ=dstQA[c].reshape(NT, P).T.copy(),
            dstW=dstWA[c].reshape(NT, P).T.copy(),
        ))
    return TW, in_maps


def run(inputs, trace=False, **kw):
    TW, in_maps = _prep(inputs)
    nc = _build(TW)
    res = run_bass_kernel_spmd(nc, in_maps, list(range(M)), trace=trace, **kw)
    out = np.concatenate([res.results[c]["out"][:NPC] for c in range(M)], axis=0)
    return out, res


def kernel(**inputs) -> np.ndarray:
    out, _ = run(inputs)
    return out



# revision 6
# speedup vs baseline: 1.3038x; 1.3038x over previous
"""GraphTransformerBlock (PyG TransformerConv + FFN block) on 8 trn2 cores.

v2: bf16 matmul pipeline, window-batched gathers + DVE ops, single ACT
table set.

Sharding: nodes partitioned into 8 contiguous ranges of 6250 (dst owner).
Edges sorted by dst on host, assigned to the owner core of their dst,
grouped into 49 windows of 128 dst nodes per core, padded to a uniform
number of 128-edge tiles per window (TW, data-dependent, computed on host).

Phase A (per window): q = x@WqT+bq and sk = x@WskipT+bskip for own nodes;
q written bf16 to a DRAM table, sk kept in SBUF.

Phase B (per window w, TW tiles of 128 edges):
  - ONE multi-offset indirect DMA gathers all TW*128 x[src] rows (bf16) and
    one more gathers all q[dst] rows (bf16) -> ~1.4us of gpsimd per gather
    instead of ~1us per tile.
  - per tile: PE-transpose x rows, kv = [k_j || m] = xsT.T@WkvT + ea.T@WeT2b
    (bf16 single-pass matmuls), ACT-copy kv PSUM->SBUF bf16, one-hot tile.
  - window-batched DVE: scr = qr*k (bf16 2x), alpha = 4D reduce,
    ex = exp(alpha/sqrt(C)) on ACT, per-tile m*ex_h scaling (bf16 4x TS),
    ex copied into the scatter rhs.
  - per tile: one scatter matmul agg += oh.T @ [m*ex || ex].
Epilogue (per window): softmax denominators commute with the segment sum;
mean heads, +skip +residual, LN1 via ACT accum stats and
rstd = exp(-0.5*ln(var+eps)) (keeps exp/ln/copy/square/prelu in ONE ACT
table set -> zero table reloads), FFN computed transposed (f1T = W1T'.T@uT)
so the leaky-relu (Prelu) bias is per-partition, LN2, write out fp32.
"""

import math

import numpy as np
import ml_dtypes

import concourse.bass as bass
import concourse.bacc as bacc
import concourse.mybir as mybir
from concourse.tile import TileContext
from concourse.bass_utils import run_bass_kernel_spmd

# Keep every activation (exp/ln/copy/square/prelu) in ONE table set.
# The set-load pass picks the first set containing each function; emptying
# the pure-exp and pure-ln entries (names and positions preserved, so set
# ids stay valid) makes natural_log_exp_and_others cover everything ->
# a single ACT_TABLE_LOAD instead of 4 per window.
import concourse.hw_specs as _hw_specs

_orig_gat = _hw_specs.get_activation_tables


def _gat_single_set(module_arch):
    t = dict(_orig_gat(module_arch))
    for name in ("exp_and_others", "natural_log"):
        if name in t and "natural_log_exp_and_others" in t:
            t[name] = set()
    return t


bacc.get_activation_tables = _gat_single_set

N, E, HID, EDIM, H, C = 50000, 400000, 128, 32, 2, 128
M = 8            # cores
NPC = N // M     # 6250 nodes per core
P = 128
NW = math.ceil(NPC / P)      # 49 windows per core
NPAD = NW * P                # 6272
NFULL = math.ceil(N / P) * P # 50048
F32 = mybir.dt.float32
BF16 = mybir.dt.bfloat16
I32 = mybir.dt.int32
INV_SQRT_C = 1.0 / math.sqrt(C)
BF = ml_dtypes.bfloat16

_cache: dict[int, "bacc.Bacc"] = {}


def _build(TW: int) -> "bacc.Bacc":
    if TW in _cache:
        return _cache[TW]
    NT = NW * TW            # edge tiles per core
    EPC = NT * P            # padded edges per core

    nc = bacc.Bacc("TRN2", target_bir_lowering=False)
    x_full = nc.declare_dram_parameter("x_full", [NFULL, HID], BF16, isOutput=False)
    x_own_b = nc.declare_dram_parameter("x_own_b", [NPAD, HID], BF16, isOutput=False)
    x_own_f = nc.declare_dram_parameter("x_own_f", [NPAD, HID], F32, isOutput=False)
    eaT = nc.declare_dram_parameter("eaT", [EDIM + 1, EPC], BF16, isOutput=False)
    srcQ = nc.declare_dram_parameter("srcQ", [P, NT], I32, isOutput=False)
    dstQ = nc.declare_dram_parameter("dstQ", [P, NT], I32, isOutput=False)
    dstW = nc.declare_dram_parameter("dstW", [P, NT], F32, isOutput=False)
    WkvT = nc.declare_dram_parameter("WkvT", [HID, 2 * H * C], BF16, isOutput=False)
    WeT2b = nc.declare_dram_parameter("WeT2b", [EDIM + 1, 2 * H * C], BF16, isOutput=False)
    WqT = nc.declare_dram_parameter("WqT", [HID, H * C], BF16, isOutput=False)
    bq_row = nc.declare_dram_parameter("bq_row", [1, H * C], BF16, isOutput=False)
    WskipT = nc.declare_dram_parameter("WskipT", [HID, HID], BF16, isOutput=False)
    bsk_row = nc.declare_dram_parameter("bsk_row", [1, HID], BF16, isOutput=False)
    W1Tg = nc.declare_dram_parameter("W1Tg", [HID, 2 * HID], BF16, isOutput=False)
    b1c = nc.declare_dram_parameter("b1c", [P, 2], F32, isOutput=False)
    W2Ta = nc.declare_dram_parameter("W2Ta", [HID, HID], BF16, isOutput=False)
    W2Tb = nc.declare_dram_parameter("W2Tb", [HID, HID], BF16, isOutput=False)
    b2row = nc.declare_dram_parameter("b2row", [1, HID], BF16, isOutput=False)
    ln1g_t = nc.declare_dram_parameter("ln1g_t", [P, HID], BF16, isOutput=False)
    ln2g_t = nc.declare_dram_parameter("ln2g_t", [P, HID], BF16, isOutput=False)
    ln2b_t = nc.declare_dram_parameter("ln2b_t", [P, HID], F32, isOutput=False)
    ident = nc.declare_dram_parameter("ident", [P, P], BF16, isOutput=False)
    iota = nc.declare_dram_parameter("iota", [P, P], BF16, isOutput=False)
    ones_col = nc.declare_dram_parameter("ones_col", [1, P], BF16, isOutput=False)
    out = nc.declare_dram_parameter("out", [NPAD, HID], F32, isOutput=True)

    q_table = nc.dram_tensor("q_table", [NPAD, H * C], BF16)

    add = mybir.AluOpType.add
    mult = mybir.AluOpType.mult
    is_equal = mybir.AluOpType.is_equal
    AX = mybir.AxisListType.X
    Exp = mybir.ActivationFunctionType.Exp
    Ln = mybir.ActivationFunctionType.Ln
    Copy = mybir.ActivationFunctionType.Copy
    Square = mybir.ActivationFunctionType.Square
    Prelu = mybir.ActivationFunctionType.Prelu

    with TileContext(nc) as tc:
        with (
            tc.tile_pool(name="const", bufs=1) as cp,
            tc.tile_pool(name="win", bufs=2) as wp,
            tc.tile_pool(name="tile", bufs=3) as ep,
            tc.tile_pool(name="epi", bufs=2) as np_,
            tc.tile_pool(name="ps_kv", bufs=2, space="PSUM") as ps_kv,
            tc.tile_pool(name="ps_agg", bufs=2, space="PSUM") as ps_agg,
            tc.tile_pool(name="ps_t", bufs=2, space="PSUM") as ps_t,
            tc.tile_pool(name="ps_misc", bufs=2, space="PSUM") as ps_misc,
        ):
            WkvT_s = cp.tile_from(WkvT[:])
            WeT2b_s = cp.tile_from(WeT2b[:])
            WqT_s = cp.tile_from(WqT[:])
            bq_s = cp.tile_from(bq_row[:])
            WskipT_s = cp.tile_from(WskipT[:])
            bsk_s = cp.tile_from(bsk_row[:])
            W1Tg_s = cp.tile_from(W1Tg[:])
            b1c_s = cp.tile_from(b1c[:])
            W2Ta_s = cp.tile_from(W2Ta[:])
            W2Tb_s = cp.tile_from(W2Tb[:])
            b2row_s = cp.tile_from(b2row[:])
            ln1g_s = cp.tile_from(ln1g_t[:])
            ln2g_s = cp.tile_from(ln2g_t[:])
            ln2b_s = cp.tile_from(ln2b_t[:])
            ident_s = cp.tile_from(ident[:])
            iota_s = cp.tile_from(iota[:])
            ones_s = cp.tile_from(ones_col[:])
            srcQ_s = cp.tile_from(srcQ[:])
            dstQ_s = cp.tile_from(dstQ[:])
            dstW_s = cp.tile_from(dstW[:])
            sk_all = cp.tile([P, NW * HID], BF16)
            eps_s = cp.tile([P, 1], F32)
            nc.vector.memset(eps_s[:], 1e-5)

            # ---- Phase A: q table + skip (own node range) ----
            for w in range(NW):
                xwb = np_.tile([P, HID], BF16, tag="xwb_a")
                nc.sync.dma_start(out=xwb[:], in_=x_own_b[w * P:(w + 1) * P, :])
                xwT_ps = ps_t.tile([P, P], BF16, tag="pst")
                nc.tensor.transpose(out=xwT_ps[:], in_=xwb[:], identity=ident_s[:])
                xwT = np_.tile([P, P], BF16, tag="xwT_a")
                nc.scalar.activation(out=xwT[:], in_=xwT_ps[:], func=Copy)
                mm_ps = ps_misc.tile([P, 4 * HID], F32, tag="psm")
                nc.tensor.matmul(out=mm_ps[:, 0:H * C], lhsT=xwT[:], rhs=WqT_s[:],
                                 start=True, stop=False)
                nc.tensor.matmul(out=mm_ps[:, 0:H * C], lhsT=ones_s[:], rhs=bq_s[:],
                                 start=False, stop=True)
                qsb = np_.tile([P, H * C], BF16, tag="qsb_a")
                nc.scalar.activation(out=qsb[:], in_=mm_ps[:, 0:H * C], func=Copy)
                nc.sync.dma_start(out=q_table[w * P:(w + 1) * P, :], in_=qsb[:])
                nc.tensor.matmul(out=mm_ps[:, 2 * HID:3 * HID], lhsT=xwT[:],
                                 rhs=WskipT_s[:], start=True, stop=False)
                nc.tensor.matmul(out=mm_ps[:, 2 * HID:3 * HID], lhsT=ones_s[:],
                                 rhs=bsk_s[:], start=False, stop=True)
                nc.scalar.activation(out=sk_all[:, w * HID:(w + 1) * HID],
                                     in_=mm_ps[:, 2 * HID:3 * HID], func=Copy)

            # ---- Phase B: edge windows ----
            for w in range(NW):
                g0 = w * TW
                xs_all = wp.tile([P, TW, HID], BF16, tag="xs")
                qr_all = wp.tile([P, TW, H * C], BF16, tag="qr")
                for t in range(TW):
                    nc.gpsimd.indirect_dma_start(
                        out=xs_all[:, t, :], out_offset=None, in_=x_full[:],
                        in_offset=bass.IndirectOffsetOnAxis(
                            ap=srcQ_s[:, g0 + t:g0 + t + 1], axis=0))
                    nc.gpsimd.indirect_dma_start(
                        out=qr_all[:, t, :], out_offset=None, in_=q_table[:],
                        in_offset=bass.IndirectOffsetOnAxis(
                            ap=dstQ_s[:, g0 + t:g0 + t + 1], axis=0))
                ea_ch = wp.tile([EDIM + 1, TW * P], BF16, tag="ea")
                nc.sync.dma_start(out=ea_ch[:], in_=eaT[:, g0 * P:(g0 + TW) * P])

                kv_all = wp.tile([P, TW, 2 * H * C], BF16, tag="kv")
                oh_all = wp.tile([P, TW, P], BF16, tag="oh")
                for t in range(TW):
                    xsT_ps = ps_t.tile([P, P], BF16, tag="pst")
                    nc.tensor.transpose(out=xsT_ps[:], in_=xs_all[:, t, :],
                                        identity=ident_s[:])
                    xsT = ep.tile([P, P], BF16, tag="xsT")
                    nc.scalar.activation(out=xsT[:], in_=xsT_ps[:], func=Copy)
                    kv_ps = ps_kv.tile([P, 2 * H * C], F32)
                    nc.tensor.matmul(out=kv_ps[:], lhsT=xsT[:], rhs=WkvT_s[:],
                                     start=True, stop=False)
                    nc.tensor.matmul(out=kv_ps[:], lhsT=ea_ch[:, t * P:(t + 1) * P],
                                     rhs=WeT2b_s[:], start=False, stop=True)
                    nc.scalar.activation(out=kv_all[:, t, :], in_=kv_ps[:], func=Copy)
                    nc.vector.tensor_scalar(
                        out=oh_all[:, t, :], in0=iota_s[:],
                        scalar1=dstW_s[:, g0 + t:g0 + t + 1],
                        scalar2=None, op0=is_equal)

                # window-batched alpha/softmax-numerator path
                scr = wp.tile([P, TW, H * C], BF16, tag="scr")
                nc.vector.tensor_tensor(out=scr[:], in0=qr_all[:],
                                        in1=kv_all[:, :, 0:H * C], op=mult)
                alpha = wp.tile([P, TW * H], F32, tag="alpha")
                nc.vector.tensor_reduce(
                    out=alpha[:],
                    in_=scr[:].rearrange("p t (h c) -> p t h c", h=H, c=C),
                    axis=AX, op=add)
                ex = wp.tile([P, TW * H], F32, tag="ex")
                nc.scalar.activation(out=ex[:], in_=alpha[:], func=Exp,
                                     scale=INV_SQRT_C)
                mex = wp.tile([P, TW, H * C + H], BF16, tag="mex")
                for t in range(TW):
                    for h in range(H):
                        nc.vector.tensor_scalar(
                            out=mex[:, t, h * C:(h + 1) * C],
                            in0=kv_all[:, t, (H + h) * C:(H + h + 1) * C],
                            scalar1=ex[:, H * t + h:H * t + h + 1],
                            scalar2=None, op0=mult)
                nc.vector.tensor_copy(
                    out=mex[:, :, H * C:H * C + H],
                    in_=ex[:].rearrange("p (t h) -> p t h", h=H))

                agg_ps = ps_agg.tile([P, H * C + H], F32)
                for t in range(TW):
                    nc.tensor.matmul(out=agg_ps[:], lhsT=oh_all[:, t, :],
                                     rhs=mex[:, t, :],
                                     start=(t == 0), stop=(t == TW - 1))

                # ---- epilogue for window w ----
                den = np_.tile([P, H], F32, tag="den")
                nc.vector.tensor_scalar_add(den[:], agg_ps[:, H * C:H * C + H], 1e-16)
                rcp = np_.tile([P, H], F32, tag="rcp")
                nc.vector.reciprocal(out=rcp[:], in_=den[:])
                a0 = np_.tile([P, HID], BF16, tag="a0")
                a1 = np_.tile([P, HID], BF16, tag="a1")
                nc.vector.tensor_scalar(out=a0[:], in0=agg_ps[:, 0:C],
                                        scalar1=rcp[:, 0:1], scalar2=1.0 / H,
                                        op0=mult, op1=mult)
                nc.vector.tensor_scalar(out=a1[:], in0=agg_ps[:, C:2 * C],
                                        scalar1=rcp[:, 1:2], scalar2=1.0 / H,
                                        op0=mult, op1=mult)
                att = np_.tile([P, HID], BF16, tag="att")
                nc.vector.tensor_tensor(out=att[:], in0=a0[:], in1=a1[:], op=add)
                s1 = np_.tile([P, HID], BF16, tag="s1")
                nc.vector.tensor_tensor(out=s1[:], in0=att[:],
                                        in1=sk_all[:, w * HID:(w + 1) * HID], op=add)
                xw = np_.tile([P, HID], F32, tag="xw")
                nc.sync.dma_start(out=xw[:], in_=x_own_f[w * P:(w + 1) * P, :])
                pre = np_.tile([P, HID], F32, tag="pre")
                nc.vector.tensor_tensor(out=pre[:], in0=xw[:], in1=s1[:], op=add)

                # LN1 stats on ACT
                scr1 = np_.tile([P, HID], BF16, tag="scr1")
                negmu = np_.tile([P, 1], F32, tag="negmu")
                nc.scalar.activation(out=scr1[:], in_=pre[:], func=Copy,
                                     scale=-1.0 / HID, accum_out=negmu[:])
                scr2 = np_.tile([P, HID], BF16, tag="scr2")
                vs = np_.tile([P, 1], F32, tag="vs")
                nc.scalar.activation(out=scr2[:], in_=pre[:], func=Square,
                                     bias=negmu[:, 0:1], scale=1.0,
                                     accum_out=vs[:])
                lnv = np_.tile([P, 1], F32, tag="lnv")
                nc.scalar.activation(out=lnv[:], in_=vs[:], func=Ln,
                                     scale=1.0 / HID, bias=eps_s[:, 0:1])
                rstd = np_.tile([P, 1], F32, tag="rstd")
                nc.scalar.activation(out=rstd[:], in_=lnv[:], func=Exp, scale=-0.5)
                cen = np_.tile([P, HID], F32, tag="cen")
                nc.vector.tensor_scalar(out=cen[:], in0=pre[:],
                                        scalar1=negmu[:, 0:1], scalar2=None,
                                        op0=add)
                u = np_.tile([P, HID], BF16, tag="u")
                nc.vector.tensor_scalar(out=u[:], in0=cen[:],
                                        scalar1=rstd[:, 0:1], scalar2=None,
                                        op0=mult)
                hres = np_.tile([P, HID], BF16, tag="hres")
                nc.vector.tensor_tensor(out=hres[:], in0=u[:], in1=ln1g_s[:], op=mult)

                # FFN, computed transposed so Prelu bias is per-partition
                uT_ps = ps_t.tile([P, P], BF16, tag="pst")
                nc.tensor.transpose(out=uT_ps[:], in_=u[:], identity=ident_s[:])
                uT = np_.tile([P, P], BF16, tag="uT")
                nc.scalar.activation(out=uT[:], in_=uT_ps[:], func=Copy)
                mf_ps = ps_misc.tile([P, 4 * HID], F32, tag="psm")
                nc.tensor.matmul(out=mf_ps[:, 0:P], lhsT=W1Tg_s[:, 0:P],
                                 rhs=uT[:], start=True, stop=True)
                nc.tensor.matmul(out=mf_ps[:, P:2 * P], lhsT=W1Tg_s[:, P:2 * P],
                                 rhs=uT[:], start=True, stop=True)
                f1a = np_.tile([P, P], BF16, tag="f1a")
                f1b = np_.tile([P, P], BF16, tag="f1b")
                nc.scalar.activation(out=f1a[:], in_=mf_ps[:, 0:P], func=Prelu,
                                     bias=b1c_s[:, 0:1], alpha=0.01)
                nc.scalar.activation(out=f1b[:], in_=mf_ps[:, P:2 * P], func=Prelu,
                                     bias=b1c_s[:, 1:2], alpha=0.01)
                nc.tensor.matmul(out=mf_ps[:, 2 * HID:3 * HID], lhsT=f1a[:],
                                 rhs=W2Ta_s[:], start=True, stop=False)
                nc.tensor.matmul(out=mf_ps[:, 2 * HID:3 * HID], lhsT=f1b[:],
                                 rhs=W2Tb_s[:], start=False, stop=False)
                nc.tensor.matmul(out=mf_ps[:, 2 * HID:3 * HID], lhsT=ones_s[:],
                                 rhs=b2row_s[:], start=False, stop=True)
                pre2 = np_.tile([P, HID], F32, tag="pre2")
                nc.vector.tensor_tensor(out=pre2[:], in0=hres[:],
                                        in1=mf_ps[:, 2 * HID:3 * HID], op=add)

                # LN2
                scr3 = np_.tile([P, HID], BF16, tag="scr3")
                negmu2 = np_.tile([P, 1], F32, tag="negmu2")
                nc.scalar.activation(out=scr3[:], in_=pre2[:], func=Copy,
                                     scale=-1.0 / HID, accum_out=negmu2[:])
                scr4 = np_.tile([P, HID], BF16, tag="scr4")
                vs2 = np_.tile([P, 1], F32, tag="vs2")
                nc.scalar.activation(out=scr4[:], in_=pre2[:], func=Square,
                                     bias=negmu2[:, 0:1], scale=1.0,
                                     accum_out=vs2[:])
                lnv2 = np_.tile([P, 1], F32, tag="lnv2")
                nc.scalar.activation(out=lnv2[:], in_=vs2[:], func=Ln,
                                     scale=1.0 / HID, bias=eps_s[:, 0:1])
                rstd2 = np_.tile([P, 1], F32, tag="rstd2")
                nc.scalar.activation(out=rstd2[:], in_=lnv2[:], func=Exp, scale=-0.5)
                cen2 = np_.tile([P, HID], F32, tag="cen2")
                nc.vector.tensor_scalar(out=cen2[:], in0=pre2[:],
                                        scalar1=negmu2[:, 0:1], scalar2=None,
                                        op0=add)
                v2 = np_.tile([P, HID], BF16, tag="v2")
                nc.vector.tensor_scalar(out=v2[:], in0=cen2[:],
                                        scalar1=rstd2[:, 0:1], scalar2=None,
                                        op0=mult)
                og = np_.tile([P, HID], BF16, tag="og")
                nc.vector.tensor_tensor(out=og[:], in0=v2[:], in1=ln2g_s[:], op=mult)
                ow = np_.tile([P, HID], F32, tag="ow")
                nc.vector.tensor_tensor(out=ow[:], in0=og[:], in1=ln2b_s[:], op=add)
                nc.sync.dma_start(out=out[w * P:(w + 1) * P, :], in_=ow[:])

    nc.compile()
    _cache[TW] = nc
    return nc


def _prep(inputs):
    x = np.asarray(inputs["x"], np.float32)
    ei = np.asarray(inputs["edge_index"])
    ea = np.asarray(inputs["edge_attr"], np.float32)
    src = ei[0].astype(np.int64)
    dst = ei[1].astype(np.int64)
    order = np.argsort(dst, kind="stable")
    src_s = src[order].astype(np.int32)
    dst_s = dst[order]
    ea_s = ea[order]

    core = (dst_s // NPC).astype(np.int64)
    local = (dst_s - core * NPC).astype(np.int32)
    winl = local // P
    dwin = (local % P).astype(np.float32)
    gid = core * NW + winl                      # non-decreasing
    starts = np.searchsorted(gid, np.arange(M * NW), side="left")
    rank = np.arange(E) - starts[gid]
    counts = np.bincount(gid, minlength=M * NW)
    TW = max(1, int(math.ceil(counts.max() / P)))
    EPC = NW * TW * P
    slot = winl.astype(np.int64) * (TW * P) + rank

    srcA = np.zeros((M, EPC), np.int32)
    dstQA = np.zeros((M, EPC), np.int32)
    dstWA = np.full((M, EPC), -1.0, np.float32)
    eaA = np.zeros((M, EPC, EDIM), np.float32)
    srcA[core, slot] = src_s
    dstQA[core, slot] = local
    dstWA[core, slot] = dwin
    eaA[core, slot] = ea_s

    x_full = np.zeros((NFULL, HID), np.float32)
    x_full[:N] = x
    ident = np.eye(P, dtype=BF)
    iota = np.broadcast_to(np.arange(P, dtype=np.float32), (P, P)).astype(BF)

    def bb(v, width):
        return np.broadcast_to(np.asarray(v, np.float32), (P, width))

    Wk, Wv, Wq, We = (np.asarray(inputs[k], np.float32) for k in ("Wk", "Wv", "Wq", "We"))
    WkvT = np.concatenate([Wk.T, Wv.T], axis=1)
    WeT2 = np.concatenate([We.T, We.T], axis=1)
    bkv = np.concatenate([np.asarray(inputs["bk"], np.float32),
                          np.asarray(inputs["bv"], np.float32)])[None, :]
    WeT2b = np.concatenate([WeT2, bkv], axis=0)
    W1 = np.asarray(inputs["W1"], np.float32)      # [256, 128]
    W2 = np.asarray(inputs["W2"], np.float32)      # [128, 256]
    b1 = np.asarray(inputs["b1"], np.float32)
    b2 = np.asarray(inputs["b2"], np.float32)
    ln1_g = np.asarray(inputs["ln1_g"], np.float32)
    ln1_b = np.asarray(inputs["ln1_b"], np.float32)
    W1T = W1.T                                     # [128, 256]
    W1Tg = ln1_g[:, None] * W1T                    # fold LN1 gamma
    b1f = b1 + ln1_b @ W1T                         # fold LN1 beta into FFN bias
    b2f = b2 + ln1_b                               # fold LN1 beta into residual
    W2T = W2.T                                     # [256, 128]

    common = dict(
        x_full=x_full.astype(BF),
        WkvT=WkvT.astype(BF), WeT2b=WeT2b.astype(BF),
        WqT=Wq.T.astype(BF), bq_row=np.asarray(inputs["bq"], np.float32)[None, :].astype(BF),
        WskipT=np.asarray(inputs["Wskip"], np.float32).T.astype(BF),
        bsk_row=np.asarray(inputs["bskip"], np.float32)[None, :].astype(BF),
        W1Tg=W1Tg.astype(BF),
        b1c=np.stack([b1f[:HID], b1f[HID:]], axis=1).astype(np.float32).copy(),
        W2Ta=W2T[:HID].astype(BF), W2Tb=W2T[HID:].astype(BF),
        b2row=b2f[None, :].astype(BF),
        ln1g_t=bb(ln1_g, HID).astype(BF),
        ln2g_t=bb(inputs["ln2_g"], HID).astype(BF),
        ln2b_t=bb(inputs["ln2_b"], HID).astype(np.float32).copy(),
        ident=ident, iota=iota,
        ones_col=np.ones((1, P), BF),
    )

    in_maps = []
    for c in range(M):
        x_own = np.zeros((NPAD, HID), np.float32)
        x_own[:NPC] = x[c * NPC:(c + 1) * NPC]
        eaT_c = np.concatenate(
            [eaA[c].T, np.ones((1, EPC), np.float32)], axis=0).astype(BF)
        NT = NW * TW
        in_maps.append(dict(
            common,
            x_own_b=x_own.astype(BF),
            x_own_f=x_own,
            eaT=eaT_c,
            srcQ=srcA[c].reshape(NT, P).T.copy(),
            dstQ=dstQA[c].reshape(NT, P).T.copy(),
            dstW=dstWA[c].reshape(NT, P).T.copy(),
        ))
    return TW, in_maps


def run(inputs, trace=False, **kw):
    TW, in_maps = _prep(inputs)
    nc = _build(TW)
    res = run_bass_kernel_spmd(nc, in_maps, list(range(M)), trace=trace, **kw)
    out = np.concatenate([res.results[c]["out"][:NPC] for c in range(M)], axis=0)
    return out, res


def kernel(**inputs) -> np.ndarray:
    out, _ = run(inputs)
    return out


# revision 8
# speedup vs baseline: 1.3320x; 1.0216x over previous
"""GraphTransformerBlock (PyG TransformerConv + FFN block) on 8 trn2 cores.

v2: bf16 matmul pipeline, window-batched gathers + DVE ops, single ACT
table set.

Sharding: nodes partitioned into 8 contiguous ranges of 6250 (dst owner).
Edges sorted by dst on host, assigned to the owner core of their dst,
grouped into 49 windows of 128 dst nodes per core, padded to a uniform
number of 128-edge tiles per window (TW, data-dependent, computed on host).

Phase A (per window): q = x@WqT+bq and sk = x@WskipT+bskip for own nodes;
q written bf16 to a DRAM table, sk kept in SBUF.

Phase B (per window w, TW tiles of 128 edges):
  - ONE multi-offset indirect DMA gathers all TW*128 x[src] rows (bf16) and
    one more gathers all q[dst] rows (bf16) -> ~1.4us of gpsimd per gather
    instead of ~1us per tile.
  - per tile: PE-transpose x rows, kv = [k_j || m] = xsT.T@WkvT + ea.T@WeT2b
    (bf16 single-pass matmuls), ACT-copy kv PSUM->SBUF bf16, one-hot tile.
  - window-batched DVE: scr = qr*k (bf16 2x), alpha = 4D reduce,
    ex = exp(alpha/sqrt(C)) on ACT, per-tile m*ex_h scaling (bf16 4x TS),
    ex copied into the scatter rhs.
  - per tile: one scatter matmul agg += oh.T @ [m*ex || ex].
Epilogue (per window): softmax denominators commute with the segment sum;
mean heads, +skip +residual, LN1 via ACT accum stats and
rstd = exp(-0.5*ln(var+eps)) (keeps exp/ln/copy/square/prelu in ONE ACT
table set -> zero table reloads), FFN computed transposed (f1T = W1T'.T@uT)
so the leaky-relu (Prelu) bias is per-partition, LN2, write out fp32.
"""

import math

import numpy as np
import ml_dtypes

import concourse.bass as bass
import concourse.bacc as bacc
import concourse.mybir as mybir
from concourse.tile import TileContext
from concourse.bass_utils import run_bass_kernel_spmd

# Keep every activation (exp/ln/copy/square/prelu) in ONE table set.
# The set-load pass picks the first set containing each function; emptying
# the pure-exp and pure-ln entries (names and positions preserved, so set
# ids stay valid) makes natural_log_exp_and_others cover everything ->
# a single ACT_TABLE_LOAD instead of 4 per window.
import concourse.hw_specs as _hw_specs

_orig_gat = _hw_specs.get_activation_tables


def _gat_single_set(module_arch):
    t = dict(_orig_gat(module_arch))
    for name in ("exp_and_others", "natural_log"):
        if name in t and "natural_log_exp_and_others" in t:
            t[name] = set()
    return t


bacc.get_activation_tables = _gat_single_set

N, E, HID, EDIM, H, C = 50000, 400000, 128, 32, 2, 128
M = 8            # cores
NPC = N // M     # 6250 nodes per core
P = 128
NW = math.ceil(NPC / P)      # 49 windows per core
NPAD = NW * P                # 6272
NFULL = math.ceil(N / P) * P # 50048
F32 = mybir.dt.float32
BF16 = mybir.dt.bfloat16
I32 = mybir.dt.int32
INV_SQRT_C = 1.0 / math.sqrt(C)
BF = ml_dtypes.bfloat16

_cache: dict[int, "bacc.Bacc"] = {}


def _build(TW: int) -> "bacc.Bacc":
    if TW in _cache:
        return _cache[TW]
    NT = NW * TW            # edge tiles per core
    EPC = NT * P            # padded edges per core

    nc = bacc.Bacc("TRN2", target_bir_lowering=False)
    x_full = nc.declare_dram_parameter("x_full", [NFULL, HID], BF16, isOutput=False)
    x_own_b = nc.declare_dram_parameter("x_own_b", [NPAD, HID], BF16, isOutput=False)
    x_own_f = nc.declare_dram_parameter("x_own_f", [NPAD, HID], F32, isOutput=False)
    eaT = nc.declare_dram_parameter("eaT", [EDIM + 1, EPC], BF16, isOutput=False)
    srcQ = nc.declare_dram_parameter("srcQ", [P, NT], I32, isOutput=False)
    oh_tab = nc.declare_dram_parameter("oh_tab", [P, EPC], BF16, isOutput=False)
    ohT_tab = nc.declare_dram_parameter("ohT_tab", [P, EPC], BF16, isOutput=False)
    WkvT = nc.declare_dram_parameter("WkvT", [HID, 2 * H * C], BF16, isOutput=False)
    WeT2b = nc.declare_dram_parameter("WeT2b", [EDIM + 1, 2 * H * C], BF16, isOutput=False)
    WqT = nc.declare_dram_parameter("WqT", [HID, H * C], BF16, isOutput=False)
    bq_row = nc.declare_dram_parameter("bq_row", [1, H * C], BF16, isOutput=False)
    WskipT = nc.declare_dram_parameter("WskipT", [HID, HID], BF16, isOutput=False)
    bsk_row = nc.declare_dram_parameter("bsk_row", [1, HID], BF16, isOutput=False)
    W1Tg = nc.declare_dram_parameter("W1Tg", [HID, 2 * HID], BF16, isOutput=False)
    b1c = nc.declare_dram_parameter("b1c", [P, 2], F32, isOutput=False)
    W2Ta = nc.declare_dram_parameter("W2Ta", [HID, HID], BF16, isOutput=False)
    W2Tb = nc.declare_dram_parameter("W2Tb", [HID, HID], BF16, isOutput=False)
    b2row = nc.declare_dram_parameter("b2row", [1, HID], BF16, isOutput=False)
    ln1g_t = nc.declare_dram_parameter("ln1g_t", [P, HID], BF16, isOutput=False)
    ln2g_t = nc.declare_dram_parameter("ln2g_t", [P, HID], BF16, isOutput=False)
    ln2b_t = nc.declare_dram_parameter("ln2b_t", [P, HID], F32, isOutput=False)
    ident = nc.declare_dram_parameter("ident", [P, P], BF16, isOutput=False)
    ones_col = nc.declare_dram_parameter("ones_col", [1, P], BF16, isOutput=False)
    out = nc.declare_dram_parameter("out", [NPAD, HID], F32, isOutput=True)

    add = mybir.AluOpType.add
    mult = mybir.AluOpType.mult
    is_equal = mybir.AluOpType.is_equal
    AX = mybir.AxisListType.X
    Exp = mybir.ActivationFunctionType.Exp
    Ln = mybir.ActivationFunctionType.Ln
    Copy = mybir.ActivationFunctionType.Copy
    Square = mybir.ActivationFunctionType.Square
    Prelu = mybir.ActivationFunctionType.Prelu

    with TileContext(nc) as tc:
        with (
            tc.tile_pool(name="const", bufs=1) as cp,
            tc.tile_pool(name="win", bufs=3) as wp,
            tc.tile_pool(name="tile", bufs=3) as ep,
            tc.tile_pool(name="epi", bufs=2) as np_,
            tc.tile_pool(name="ps_kv", bufs=2, space="PSUM") as ps_kv,
            tc.tile_pool(name="ps_agg", bufs=1, space="PSUM") as ps_agg,
            tc.tile_pool(name="ps_t", bufs=2, space="PSUM") as ps_t,
            tc.tile_pool(name="ps_misc", bufs=1, space="PSUM") as ps_misc,
        ):
            WkvT_s = cp.tile_from(WkvT[:])
            WeT2b_s = cp.tile_from(WeT2b[:])
            WqT_s = cp.tile_from(WqT[:])
            bq_s = cp.tile_from(bq_row[:])
            WskipT_s = cp.tile_from(WskipT[:])
            bsk_s = cp.tile_from(bsk_row[:])
            W1Tg_s = cp.tile_from(W1Tg[:])
            b1c_s = cp.tile_from(b1c[:])
            W2Ta_s = cp.tile_from(W2Ta[:])
            W2Tb_s = cp.tile_from(W2Tb[:])
            b2row_s = cp.tile_from(b2row[:])
            ln1g_s = cp.tile_from(ln1g_t[:])
            ln2g_s = cp.tile_from(ln2g_t[:])
            ln2b_s = cp.tile_from(ln2b_t[:])
            ident_s = cp.tile_from(ident[:])
            ones_s = cp.tile_from(ones_col[:])
            srcQ_s = cp.tile_from(srcQ[:])
            sk_all = cp.tile([P, NW * HID], BF16)
            q_all = cp.tile([P, NW * H * C], BF16)
            eps_s = cp.tile([P, 1], F32)
            nc.vector.memset(eps_s[:], 1e-5)

            # ---- Phase A: q table + skip (own node range) ----
            for w in range(NW):
                xwb = np_.tile([P, HID], BF16, tag="xwb_a")
                nc.sync.dma_start(out=xwb[:], in_=x_own_b[w * P:(w + 1) * P, :])
                xwT_ps = ps_t.tile([P, P], BF16, tag="pst")
                nc.tensor.transpose(out=xwT_ps[:], in_=xwb[:], identity=ident_s[:])
                xwT = np_.tile([P, P], BF16, tag="xwT_a")
                nc.scalar.activation(out=xwT[:], in_=xwT_ps[:], func=Copy)
                mm_ps = ps_misc.tile([P, 4 * HID], F32, tag="psm")
                nc.tensor.matmul(out=mm_ps[:, 0:H * C], lhsT=xwT[:], rhs=WqT_s[:],
                                 start=True, stop=False)
                nc.tensor.matmul(out=mm_ps[:, 0:H * C], lhsT=ones_s[:], rhs=bq_s[:],
                                 start=False, stop=True)
                nc.scalar.activation(out=q_all[:, w * H * C:(w + 1) * H * C],
                                     in_=mm_ps[:, 0:H * C], func=Copy)
                nc.tensor.matmul(out=mm_ps[:, 2 * HID:3 * HID], lhsT=xwT[:],
                                 rhs=WskipT_s[:], start=True, stop=False)
                nc.tensor.matmul(out=mm_ps[:, 2 * HID:3 * HID], lhsT=ones_s[:],
                                 rhs=bsk_s[:], start=False, stop=True)
                nc.scalar.activation(out=sk_all[:, w * HID:(w + 1) * HID],
                                     in_=mm_ps[:, 2 * HID:3 * HID], func=Copy)

            # ---- Phase B: edge windows ----
            for w in range(NW):
                g0 = w * TW
                xs_all = wp.tile([P, TW, HID], BF16, tag="xs")
                for t in range(TW):
                    nc.gpsimd.indirect_dma_start(
                        out=xs_all[:, t, :], out_offset=None, in_=x_full[:],
                        in_offset=bass.IndirectOffsetOnAxis(
                            ap=srcQ_s[:, g0 + t:g0 + t + 1], axis=0))
                ea_ch = wp.tile([EDIM + 1, TW * P], BF16, tag="ea")
                nc.sync.dma_start(out=ea_ch[:], in_=eaT[:, g0 * P:(g0 + TW) * P])
                oh_all = wp.tile([P, TW, P], BF16, tag="oh")
                nc.sync.dma_start(out=oh_all[:], in_=oh_tab[:, g0 * P:(g0 + TW) * P])
                ohT_all = wp.tile([P, TW, P], BF16, tag="ohT")
                nc.sync.dma_start(out=ohT_all[:], in_=ohT_tab[:, g0 * P:(g0 + TW) * P])
                q_w = q_all[:, w * H * C:(w + 1) * H * C]

                kv_all = wp.tile([P, TW, 2 * H * C], BF16, tag="kv")
                scr = wp.tile([P, TW, H * C], BF16, tag="scr")
                for t in range(TW):
                    xsT_ps = ps_t.tile([P, P], BF16, tag="pst")
                    nc.tensor.transpose(out=xsT_ps[:], in_=xs_all[:, t, :],
                                        identity=ident_s[:])
                    xsT = ep.tile([P, P], BF16, tag="xsT")
                    nc.scalar.activation(out=xsT[:], in_=xsT_ps[:], func=Copy)
                    kv_ps = ps_kv.tile([P, 2 * H * C + H * C], F32)
                    nc.tensor.matmul(out=kv_ps[:, 0:2 * H * C], lhsT=xsT[:],
                                     rhs=WkvT_s[:], start=True, stop=False)
                    nc.tensor.matmul(out=kv_ps[:, 0:2 * H * C],
                                     lhsT=ea_ch[:, t * P:(t + 1) * P],
                                     rhs=WeT2b_s[:], start=False, stop=True)
                    nc.scalar.activation(out=kv_all[:, t, :],
                                         in_=kv_ps[:, 0:2 * H * C], func=Copy)
                    # qr = per-edge q rows selected from the window q block
                    nc.tensor.matmul(out=kv_ps[:, 2 * H * C:3 * H * C],
                                     lhsT=ohT_all[:, t, :], rhs=q_w,
                                     start=True, stop=True)
                    nc.vector.tensor_tensor(out=scr[:, t, :],
                                            in0=kv_ps[:, 2 * H * C:3 * H * C],
                                            in1=kv_all[:, t, 0:H * C], op=mult)

                alpha = wp.tile([P, TW * H], F32, tag="alpha")
                nc.vector.tensor_reduce(
                    out=alpha[:],
                    in_=scr[:].rearrange("p t (h c) -> p t h c", h=H, c=C),
                    axis=AX, op=add)
                ex = wp.tile([P, TW * H], F32, tag="ex")
                nc.scalar.activation(out=ex[:], in_=alpha[:], func=Exp,
                                     scale=INV_SQRT_C)
                mex = wp.tile([P, TW, H * C + H], BF16, tag="mex")
                for t in range(TW):
                    for h in range(H):
                        nc.vector.tensor_scalar(
                            out=mex[:, t, h * C:(h + 1) * C],
                            in0=kv_all[:, t, (H + h) * C:(H + h + 1) * C],
                            scalar1=ex[:, H * t + h:H * t + h + 1],
                            scalar2=None, op0=mult)
                nc.vector.tensor_copy(
                    out=mex[:, :, H * C:H * C + H],
                    in_=ex[:].rearrange("p (t h) -> p t h", h=H))

                agg_ps = ps_agg.tile([P, H * C + H], F32)
                for t in range(TW):
                    nc.tensor.matmul(out=agg_ps[:], lhsT=oh_all[:, t, :],
                                     rhs=mex[:, t, :],
                                     start=(t == 0), stop=(t == TW - 1))

                # ---- epilogue for window w ----
                den = np_.tile([P, H], F32, tag="den")
                nc.vector.tensor_scalar_add(den[:], agg_ps[:, H * C:H * C + H], 1e-16)
                rcp = np_.tile([P, H], F32, tag="rcp")
                nc.vector.reciprocal(out=rcp[:], in_=den[:])
                a0 = np_.tile([P, HID], BF16, tag="a0")
                a1 = np_.tile([P, HID], BF16, tag="a1")
                nc.vector.tensor_scalar(out=a0[:], in0=agg_ps[:, 0:C],
                                        scalar1=rcp[:, 0:1], scalar2=1.0 / H,
                                        op0=mult, op1=mult)
                nc.vector.tensor_scalar(out=a1[:], in0=agg_ps[:, C:2 * C],
                                        scalar1=rcp[:, 1:2], scalar2=1.0 / H,
                                        op0=mult, op1=mult)
                att = np_.tile([P, HID], BF16, tag="att")
                nc.vector.tensor_tensor(out=att[:], in0=a0[:], in1=a1[:], op=add)
                s1 = np_.tile([P, HID], BF16, tag="s1")
                nc.vector.tensor_tensor(out=s1[:], in0=att[:],
                                        in1=sk_all[:, w * HID:(w + 1) * HID], op=add)
                xw = np_.tile([P, HID], F32, tag="xw")
                nc.sync.dma_start(out=xw[:], in_=x_own_f[w * P:(w + 1) * P, :])
                pre = np_.tile([P, HID], F32, tag="pre")
                nc.vector.tensor_tensor(out=pre[:], in0=xw[:], in1=s1[:], op=add)

                # LN1 stats on ACT
                scr1 = np_.tile([P, HID], BF16, tag="scr1")
                negmu = np_.tile([P, 1], F32, tag="negmu")
                nc.scalar.activation(out=scr1[:], in_=pre[:], func=Copy,
                                     scale=-1.0 / HID, accum_out=negmu[:])
                scr2 = np_.tile([P, HID], BF16, tag="scr2")
                vs = np_.tile([P, 1], F32, tag="vs")
                nc.scalar.activation(out=scr2[:], in_=pre[:], func=Square,
                                     bias=negmu[:, 0:1], scale=1.0,
                                     accum_out=vs[:])
                lnv = np_.tile([P, 1], F32, tag="lnv")
                nc.scalar.activation(out=lnv[:], in_=vs[:], func=Ln,
                                     scale=1.0 / HID, bias=eps_s[:, 0:1])
                rstd = np_.tile([P, 1], F32, tag="rstd")
                nc.scalar.activation(out=rstd[:], in_=lnv[:], func=Exp, scale=-0.5)
                cen = np_.tile([P, HID], F32, tag="cen")
                nc.vector.tensor_scalar(out=cen[:], in0=pre[:],
                                        scalar1=negmu[:, 0:1], scalar2=None,
                                        op0=add)
                u = np_.tile([P, HID], BF16, tag="u")
                nc.vector.tensor_scalar(out=u[:], in0=cen[:],
                                        scalar1=rstd[:, 0:1], scalar2=None,
                                        op0=mult)
                hres = np_.tile([P, HID], BF16, tag="hres")
                nc.vector.tensor_tensor(out=hres[:], in0=u[:], in1=ln1g_s[:], op=mult)

                # FFN, computed transposed so Prelu bias is per-partition
                uT_ps = ps_t.tile([P, P], BF16, tag="pst")
                nc.tensor.transpose(out=uT_ps[:], in_=u[:], identity=ident_s[:])
                uT = np_.tile([P, P], BF16, tag="uT")
                nc.scalar.activation(out=uT[:], in_=uT_ps[:], func=Copy)
                mf_ps = ps_misc.tile([P, 4 * HID], F32, tag="psm")
                nc.tensor.matmul(out=mf_ps[:, 0:P], lhsT=W1Tg_s[:, 0:P],
                                 rhs=uT[:], start=True, stop=True)
                nc.tensor.matmul(out=mf_ps[:, P:2 * P], lhsT=W1Tg_s[:, P:2 * P],
                                 rhs=uT[:], start=True, stop=True)
                f1a = np_.tile([P, P], BF16, tag="f1a")
                f1b = np_.tile([P, P], BF16, tag="f1b")
                nc.scalar.activation(out=f1a[:], in_=mf_ps[:, 0:P], func=Prelu,
                                     bias=b1c_s[:, 0:1], alpha=0.01)
                nc.scalar.activation(out=f1b[:], in_=mf_ps[:, P:2 * P], func=Prelu,
                                     bias=b1c_s[:, 1:2], alpha=0.01)
                nc.tensor.matmul(out=mf_ps[:, 2 * HID:3 * HID], lhsT=f1a[:],
                                 rhs=W2Ta_s[:], start=True, stop=False)
                nc.tensor.matmul(out=mf_ps[:, 2 * HID:3 * HID], lhsT=f1b[:],
                                 rhs=W2Tb_s[:], start=False, stop=False)
                nc.tensor.matmul(out=mf_ps[:, 2 * HID:3 * HID], lhsT=ones_s[:],
                                 rhs=b2row_s[:], start=False, stop=True)
                pre2 = np_.tile([P, HID], F32, tag="pre2")
                nc.vector.tensor_tensor(out=pre2[:], in0=hres[:],
                                        in1=mf_ps[:, 2 * HID:3 * HID], op=add)

                # LN2
                scr3 = np_.tile([P, HID], BF16, tag="scr3")
                negmu2 = np_.tile([P, 1], F32, tag="negmu2")
                nc.scalar.activation(out=scr3[:], in_=pre2[:], func=Copy,
                                     scale=-1.0 / HID, accum_out=negmu2[:])
                scr4 = np_.tile([P, HID], BF16, tag="scr4")
                vs2 = np_.tile([P, 1], F32, tag="vs2")
                nc.scalar.activation(out=scr4[:], in_=pre2[:], func=Square,
                                     bias=negmu2[:, 0:1], scale=1.0,
                                     accum_out=vs2[:])
                lnv2 = np_.tile([P, 1], F32, tag="lnv2")
                nc.scalar.activation(out=lnv2[:], in_=vs2[:], func=Ln,
                                     scale=1.0 / HID, bias=eps_s[:, 0:1])
                rstd2 = np_.tile([P, 1], F32, tag="rstd2")
                nc.scalar.activation(out=rstd2[:], in_=lnv2[:], func=Exp, scale=-0.5)
                cen2 = np_.tile([P, HID], F32, tag="cen2")
                nc.vector.tensor_scalar(out=cen2[:], in0=pre2[:],
                                        scalar1=negmu2[:, 0:1], scalar2=None,
                                        op0=add)
                v2 = np_.tile([P, HID], BF16, tag="v2")
                nc.vector.tensor_scalar(out=v2[:], in0=cen2[:],
                                        scalar1=rstd2[:, 0:1], scalar2=None,
                                        op0=mult)
                og = np_.tile([P, HID], BF16, tag="og")
                nc.vector.tensor_tensor(out=og[:], in0=v2[:], in1=ln2g_s[:], op=mult)
                ow = np_.tile([P, HID], F32, tag="ow")
                nc.vector.tensor_tensor(out=ow[:], in0=og[:], in1=ln2b_s[:], op=add)
                nc.sync.dma_start(out=out[w * P:(w + 1) * P, :], in_=ow[:])

    nc.compile()
    _cache[TW] = nc
    return nc


def _prep(inputs):
    x = np.asarray(inputs["x"], np.float32)
    ei = np.asarray(inputs["edge_index"])
    ea = np.asarray(inputs["edge_attr"], np.float32)
    src = ei[0].astype(np.int64)
    dst = ei[1].astype(np.int64)
    order = np.argsort(dst, kind="stable")
    src_s = src[order].astype(np.int32)
    dst_s = dst[order]
    ea_s = ea[order]

    core = (dst_s // NPC).astype(np.int64)
    local = (dst_s - core * NPC).astype(np.int32)
    winl = local // P
    dwin = (local % P).astype(np.float32)
    gid = core * NW + winl                      # non-decreasing
    starts = np.searchsorted(gid, np.arange(M * NW), side="left")
    rank = np.arange(E) - starts[gid]
    counts = np.bincount(gid, minlength=M * NW)
    TW = max(1, int(math.ceil(counts.max() / P)))
    EPC = NW * TW * P
    slot = winl.astype(np.int64) * (TW * P) + rank

    srcA = np.zeros((M, EPC), np.int32)
    dstQA = np.zeros((M, EPC), np.int32)
    dstWA = np.full((M, EPC), -1.0, np.float32)
    eaA = np.zeros((M, EPC, EDIM), np.float32)
    srcA[core, slot] = src_s
    dstQA[core, slot] = local
    dstWA[core, slot] = dwin
    eaA[core, slot] = ea_s

    x_full = np.zeros((NFULL, HID), np.float32)
    x_full[:N] = x
    ident = np.eye(P, dtype=BF)

    def bb(v, width):
        return np.broadcast_to(np.asarray(v, np.float32), (P, width))

    Wk, Wv, Wq, We = (np.asarray(inputs[k], np.float32) for k in ("Wk", "Wv", "Wq", "We"))
    WkvT = np.concatenate([Wk.T, Wv.T], axis=1)
    WeT2 = np.concatenate([We.T, We.T], axis=1)
    bkv = np.concatenate([np.asarray(inputs["bk"], np.float32),
                          np.asarray(inputs["bv"], np.float32)])[None, :]
    WeT2b = np.concatenate([WeT2, bkv], axis=0)
    W1 = np.asarray(inputs["W1"], np.float32)      # [256, 128]
    W2 = np.asarray(inputs["W2"], np.float32)      # [128, 256]
    b1 = np.asarray(inputs["b1"], np.float32)
    b2 = np.asarray(inputs["b2"], np.float32)
    ln1_g = np.asarray(inputs["ln1_g"], np.float32)
    ln1_b = np.asarray(inputs["ln1_b"], np.float32)
    W1T = W1.T                                     # [128, 256]
    W1Tg = ln1_g[:, None] * W1T                    # fold LN1 gamma
    b1f = b1 + ln1_b @ W1T                         # fold LN1 beta into FFN bias
    b2f = b2 + ln1_b                               # fold LN1 beta into residual
    W2T = W2.T                                     # [256, 128]

    common = dict(
        x_full=x_full.astype(BF),
        WkvT=WkvT.astype(BF), WeT2b=WeT2b.astype(BF),
        WqT=Wq.T.astype(BF), bq_row=np.asarray(inputs["bq"], np.float32)[None, :].astype(BF),
        WskipT=np.asarray(inputs["Wskip"], np.float32).T.astype(BF),
        bsk_row=np.asarray(inputs["bskip"], np.float32)[None, :].astype(BF),
        W1Tg=W1Tg.astype(BF),
        b1c=np.stack([b1f[:HID], b1f[HID:]], axis=1).astype(np.float32).copy(),
        W2Ta=W2T[:HID].astype(BF), W2Tb=W2T[HID:].astype(BF),
        b2row=b2f[None, :].astype(BF),
        ln1g_t=bb(ln1_g, HID).astype(BF),
        ln2g_t=bb(inputs["ln2_g"], HID).astype(BF),
        ln2b_t=bb(inputs["ln2_b"], HID).astype(np.float32).copy(),
        ident=ident,
        ones_col=np.ones((1, P), BF),
    )

    in_maps = []
    NT = NW * TW
    for c in range(M):
        x_own = np.zeros((NPAD, HID), np.float32)
        x_own[:NPC] = x[c * NPC:(c + 1) * NPC]
        eaT_c = np.concatenate(
            [eaA[c].T, np.ones((1, EPC), np.float32)], axis=0).astype(BF)
        dw = dstWA[c].reshape(NT, P)                 # slot (g, p) -> local dst
        gs, ps = np.nonzero(dw >= 0)
        ds = dw[gs, ps].astype(np.int64)
        oh_c = np.zeros((P, NT, P), BF)              # [edge p, tile, dst n]
        oh_c[ps, gs, ds] = 1
        ohT_c = np.zeros((P, NT, P), BF)             # [dst n, tile, edge e]
        ohT_c[ds, gs, ps] = 1
        in_maps.append(dict(
            common,
            x_own_b=x_own.astype(BF),
            x_own_f=x_own,
            eaT=eaT_c,
            srcQ=srcA[c].reshape(NT, P).T.copy(),
            oh_tab=oh_c.reshape(P, EPC),
            ohT_tab=ohT_c.reshape(P, EPC),
        ))
    return TW, in_maps


def run(inputs, trace=False, **kw):
    TW, in_maps = _prep(inputs)
    nc = _build(TW)
    res = run_bass_kernel_spmd(nc, in_maps, list(range(M)), trace=trace, **kw)
    out = np.concatenate([res.results[c]["out"][:NPC] for c in range(M)], axis=0)
    return out, res


def kernel(**inputs) -> np.ndarray:
    out, _ = run(inputs)
    return out
